# revision 62
# baseline (speedup 1.0000x reference)
"""Trainium2 Bass kernel for a 2-layer GAT (nn_LogicGNN): 8-core SPMD.

Sharding: destination nodes across 8 cores (each core owns N/8 dst nodes and
all edges into them -> softmax stats are core-local, no all-reduce). Dense
projections are node-sharded and exchanged with AllGather. Edge phase: per
128-dst-node block, one dense self-loop tile plus packed 128-edge dma_gather
tiles; softmax computed without max-subtraction (logits are O(1) here,
mathematically identical); one PSUM matmul per tile against a 0/1 selection
matrix accumulates softmax denominators and weighted feature sums together.

V2: tables/edge pipeline in bf16 (half the gather bytes, 2x DVE/PE rate);
dst-side logits come from a host-prebaked transposed selection matrix (stf)
streamed sequentially + one small matmul per tile, replacing the per-edge
dst-logit dma_gather (which was index-rate bound on gpsimd) and L2's
per-tile PE transposes; leaky-relu moved to the scalar engine (Lrelu).
"""
import sys
sys.path.insert(0, "/opt/trn_rl_repo")
sys.path.insert(0, "/root/.axon_site")

import numpy as np
import ml_dtypes

BF16 = np.float16

N = 50000
E = 800000
IN_F, HID, OUT_F, HEADS = 128, 64, 128, 4
NEG_SLOPE = 0.2
N_CORES = 8
LOC = 6250                    # real nodes per core
LOCP = 6272                   # padded to 49*128
NBLK = LOCP // 128            # 49 blocks per core
NT = N_CORES * LOCP           # table rows = 50176
HALF = NT // 2                # 25088, int16-safe gather halves
ROW1 = 384                    # L1 table row bf16 elems (768B): [h1 256|as 4|pad]
ROW2 = 256                    # L2 table row bf16 elems (512B): [h2 128|as2 1|pad]
COL1 = 264                    # own1 cols: h 256 | as 4 | ad 4
GCOL1 = 260                   # gathered cols used: h 256 | as 4
COL2 = 130                    # own2 cols: h2 128 | as2 1 | ad2 1
GCOL2 = 129
GB = 8                        # tiles per dma_gather call (1024 idxs)
EPS = 1e-30
AG_SIZES = [1568, 1568, 1568, 1568]        # AllGather chunk rows
AG_STARTS = [0, 1568, 3136, 4704]
AG_INBASE = [0, 12544, 0, 12544]           # output base within the half-table
AG_ISB = [0, 0, 1, 1]                      # chunk belongs to half B
AG_BLKS = [12, 24, 36, 48]                 # last producing block per chunk

_cache = {}


def _plan(edge_index):
    """Host preprocessing. Returns the shared tile plan [(block, half)...] and
    per-core idx16 [C,T,128] (row index within table half) + dstrow [C,T,128]
    (dst position within the 128-node block; 999 for pad lanes)."""
    src = np.concatenate([edge_index[0], np.arange(N, dtype=np.int64)])
    dst = np.concatenate([edge_index[1], np.arange(N, dtype=np.int64)])
    is_added_loop = np.zeros(len(src), dtype=bool)
    is_added_loop[E:] = True                 # only the appended loops go dense
    owner = dst // LOC
    o_ = src // LOC
    l_ = src % LOC
    # chunk-major half-table layout (half A = local rows [0,3136)): each
    # AllGather chunk's output region is contiguous; the last chunk is small
    # so the final ship after the producing phase's tail is cheap
    sz = np.array(AG_SIZES); st = np.array(AG_STARTS)
    ib = np.array(AG_INBASE); isb = np.array(AG_ISB)
    ends = np.cumsum(sz)
    c_ = np.searchsorted(ends, l_, side='right')
    trow = isb[c_] * HALF + ib[c_] + o_ * sz[c_] + (l_ - st[c_])

    per_core = []
    cnt = np.zeros((N_CORES, NBLK, 2), dtype=np.int64)
    for c in range(N_CORES):
        m = (owner == c) & (~is_added_loop)
        ld = (dst[m] - c * LOC).astype(np.int64)
        tr = trow[m]
        blk = ld // 128
        half = (tr >= HALF).astype(np.int64)
        order = np.lexsort((ld, half, blk))
        ld, tr, blk, half = ld[order], tr[order], blk[order], half[order]
        per_core.append((ld, tr, blk, half))
        for b in range(NBLK):
            mb = blk == b
            cnt[c, b, 0] = np.count_nonzero(mb & (half == 0))
            cnt[c, b, 1] = np.count_nonzero(mb & (half == 1))
    tiles = np.ceil(cnt / 128.0).astype(np.int64).max(axis=0)   # [NBLK, 2]

    plan = []
    for b in range(NBLK):
        for h in (0, 1):
            plan.extend([(b, h)] * int(tiles[b, h]))
    Ttot = len(plan)
    idx16 = np.zeros((N_CORES, Ttot, 128), dtype=np.int16)
    dstrow = np.full((N_CORES, Ttot, 128), 999.0, dtype=np.float32)
    for c in range(N_CORES):
        ld, tr, blk, half = per_core[c]
        ti = 0
        for b in range(NBLK):
            for h in (0, 1):
                m = (blk == b) & (half == h)
                lds, trs = ld[m], tr[m]
                k = len(lds)
                for _t in range(int(tiles[b, h])):
                    lo = _t * 128
                    n_here = max(0, min(128, k - lo))
                    if n_here > 0:
                        rel = trs[lo:lo + n_here] - (HALF if h else 0)
                        idx16[c, ti, :n_here] = rel.astype(np.int16)
                        dstrow[c, ti, :n_here] = (
                            lds[lo:lo + n_here] - b * 128).astype(np.float32)
                    ti += 1
    return plan, idx16, dstrow


def _wrap16(idx):
    """[T,128] int16 -> dma_gather wrapped idx layout [128, T*8]."""
    T = idx.shape[0]
    out = np.zeros((128, T * 8), dtype=np.int16)
    for t in range(T):
        blk = idx[t].reshape(8, 16).T
        out[:, t * 8:(t + 1) * 8] = np.tile(blk, (8, 1))
    return out


def _build(plan):
    import concourse.bacc as bacc
    import concourse.mybir as mybir
    from concourse import tile
    from concourse.library_config import mlp

    f32 = mybir.dt.float32
    bf16 = mybir.dt.float16
    Ttot = len(plan)

    nc = bacc.Bacc("TRN2", target_bir_lowering=False, debug=False,
                   num_devices=N_CORES, num_swdge_queues=4)

    xT = nc.dram_tensor("xT", [IN_F, NT], bf16, kind="ExternalInput")
    xTo = nc.dram_tensor("xTo", [IN_F, LOCP], bf16, kind="ExternalInput")
    wcat = nc.dram_tensor("wcat", [IN_F, COL1], bf16, kind="ExternalInput")
    w2a = nc.dram_tensor("w2a", [HEADS * HID, COL2], bf16, kind="ExternalInput")
    b1row = nc.dram_tensor("b1row", [128, 256], f32, kind="ExternalInput")
    s4f_d = nc.dram_tensor("s4f", [128, Ttot * 128], mybir.dt.float8e4, kind="ExternalInput")
    ident = nc.dram_tensor("ident", [128, 128], bf16, kind="ExternalInput")
    identf = nc.dram_tensor("identf", [128, 128], f32, kind="ExternalInput")
    idx_d = nc.dram_tensor("idx", [128, Ttot * 8], mybir.dt.int16, kind="ExternalInput")
    stf_d = nc.dram_tensor("stf", [128, Ttot * 128], mybir.dt.float8e4, kind="ExternalInput")
    out_d = nc.dram_tensor("out", [LOCP, OUT_F], f32, kind="ExternalOutput")

    l1tabA = nc.dram_tensor("l1tabA", [HALF, ROW1], bf16)
    l1tabB = nc.dram_tensor("l1tabB", [HALF, ROW1], bf16)
    l2sh = nc.dram_tensor("l2sh", [LOCP, ROW2], bf16)
    l2tabA = nc.dram_tensor("l2tabA", [HALF, ROW2], bf16, addr_space="Shared")
    l2tabB = nc.dram_tensor("l2tabB", [HALF, ROW2], bf16, addr_space="Shared")
    own1 = nc.dram_tensor("own1", [LOCP, COL1], bf16)
    own2 = nc.dram_tensor("own2", [LOCP, COL2], bf16)

    with tile.TileContext(nc) as tc:
        nc.gpsimd.load_library(mlp)
        with (
            tc.tile_pool(name="const", bufs=1) as cp,
            tc.tile_pool(name="io", bufs=6) as iop,
            tc.tile_pool(name="g", bufs=14) as gp,
            tc.tile_pool(name="stg", bufs=10) as sgp,
            tc.tile_pool(name="s4", bufs=10) as s4p,
            tc.tile_pool(name="work", bufs=3) as wp,
            tc.tile_pool(name="selfg", bufs=8) as sp,
            tc.tile_pool(name="blk", bufs=2) as bp,
            tc.tile_pool(name="ps", bufs=2, space="PSUM") as pp,
            tc.tile_pool(name="psh2", bufs=2, space="PSUM") as ph,
            tc.tile_pool(name="psu", bufs=2, space="PSUM") as pu,
            tc.tile_pool(name="psad", bufs=2, space="PSUM") as pa,
        ):
            wc = cp.tile([128, COL1], bf16)
            nc.sync.dma_start(wc[:], wcat[:])
            w2c = cp.tile([128, 2 * COL2], bf16)
            nc.sync.dma_start(w2c[:, :COL2], w2a[0:128, :])
            nc.sync.dma_start(w2c[:, COL2:], w2a[128:256, :])
            b1t = cp.tile([128, 256], f32)
            nc.sync.dma_start(b1t[:], b1row[:])
            c02 = cp.tile([128, 2], bf16)
            nc.vector.memset(c02[:], NEG_SLOPE)
            cm1 = cp.tile([128, 2], f32)
            nc.vector.memset(cm1[:], 1.0)

            idt = cp.tile([128, 128], bf16)
            nc.sync.dma_start(idt[:], ident[:])
            idtf = cp.tile([128, 128], f32)
            nc.sync.dma_start(idtf[:], identf[:])
            idxs = cp.tile([128, Ttot * 8], mybir.dt.int16)
            nc.sync.dma_start(idxs[:], idx_d[:])


            # ---------- P0 ----------
            def ag_chunk(sh, tabA_, tabB_, k, gcol):
                r0, rows = AG_STARTS[k], AG_SIZES[k]
                tab = tabB_ if AG_ISB[k] else tabA_
                ob = AG_INBASE[k]
                nc.gpsimd.collective_compute(
                    "AllGather", mybir.AluOpType.bypass,
                    ins=[sh[r0:r0 + rows, :]],
                    outs=[tab[ob:ob + 8 * rows, :]],
                    replica_groups=[list(range(N_CORES))],
                )

            # Redundant full-table build: every core computes the whole L1
            # table locally (x is replicated), so there is no layer-1
            # AllGather and half-A gathers can start once half A is written.
            # j-major order: all 8 cores' block j together, so half A
            # (l < 3136) completes at the 50% mark.
            def tab_segments(l0, l1_):
                """split local-row range [l0,l1) at chunk boundaries ->
                (tab_is_b, table_row0, l_off, n)"""
                segs = []
                for k2 in range(len(AG_SIZES)):
                    a = max(l0, AG_STARTS[k2])
                    b2 = min(l1_, AG_STARTS[k2] + AG_SIZES[k2])
                    if a < b2:
                        segs.append((AG_ISB[k2],
                                     AG_INBASE[k2] + (a - AG_STARTS[k2]),
                                     a - l0, b2 - a, AG_SIZES[k2]))
                return segs

            for j in range(NBLK):
                xtj = iop.tile([128, 1024], bf16, tag="xt")
                nc.sync.dma_start(xtj[:], xT[:, j * 1024:(j + 1) * 1024])
                for o in range(N_CORES):
                    ps = pp.tile([128, COL1], f32, tag="scratch")
                    nc.tensor.matmul(ps[:], lhsT=xtj[:, o * 128:(o + 1) * 128],
                                     rhs=wc[:], start=True, stop=True)
                    hrow = iop.tile([128, COL1], bf16, tag="hrow")
                    nc.scalar.copy(hrow[:], ps[:])
                    for (is_b, tr0, poff, n, csz) in tab_segments(
                            j * 128, (j + 1) * 128):
                        tab = l1tabB if is_b else l1tabA
                        r = tr0 + o * csz
                        nc.scalar.dma_start(tab[r:r + n, 0:COL1],
                                            hrow[poff:poff + n, :])

            # own rows for the self path: this core's own 49 blocks
            xfull = cp.tile([128, LOCP], bf16)
            nc.sync.dma_start(xfull[:], xTo[:])
            for j in range(NBLK):
                ps = pp.tile([128, COL1], f32, tag="scratch")
                nc.tensor.matmul(ps[:], lhsT=xfull[:, j * 128:(j + 1) * 128],
                                 rhs=wc[:], start=True, stop=True)
                hrow = iop.tile([128, COL1], bf16, tag="hrow")
                nc.scalar.copy(hrow[:], ps[:])
                nc.scalar.dma_start(own1[j * 128:(j + 1) * 128, 0:COL1],
                                    hrow[:])

            def edge_layer(tabA, tabB, ownt, rowlen, colown, gcol, nheads,
                           fdim, finish_block, after_block=None):
                # group spans per block: [(s, k, half), ...]
                spans = [[] for _ in range(NBLK)]
                t0 = 0
                while t0 < Ttot:
                    b0, h0 = plan[t0]
                    t1 = t0
                    while t1 < Ttot and plan[t1] == (b0, h0):
                        t1 += 1
                    for s in range(t0, t1, GB):
                        spans[b0].append((s, min(s + GB, t1) - s, h0))
                    t0 = t1

                NGMAX = max(len(sp_) for sp_ in spans)
                assert NGMAX * GB * nheads <= 512, (NGMAX, nheads)
                qi = [0]
                PF = 6

                def emit_block(b):
                    selfG = sp.tile([128, COL1], bf16, tag="sg")
                    nc.sync.dma_start(selfG[:, :colown],
                                      ownt[b * 128:(b + 1) * 128, 0:colown])
                    groups = []
                    for (s, k, h0) in spans[b]:
                        gt_raw = gp.tile([128, GB * ROW1], bf16, tag="g")
                        gt = gt_raw[:, :k * rowlen].rearrange(
                            "p (t r) -> p t r", r=rowlen)
                        nc.gpsimd.dma_gather(
                            out_ap=gt,
                            in_ap=tabB[:] if h0 else tabA[:],
                            idxs_ap=idxs[:, s * 8:(s + k) * 8],
                            num_idxs=128 * k, num_idxs_reg=128 * k,
                            elem_size=rowlen, queue_num=qi[0] % 4)
                        qi[0] += 1
                        stg = sgp.tile([128, GB * 128], mybir.dt.float8e4, tag="stg")
                        nc.sync.dma_start(stg[:, :k * 128],
                                          stf_d[:, s * 128:(s + k) * 128])
                        s4 = s4p.tile([128, GB * 128], mybir.dt.float8e4, tag="S4")
                        nc.sync.dma_start(s4[:, :k * 128],
                                          s4f_d[:, s * 128:(s + k) * 128])
                        groups.append((s, k, gt_raw, stg, s4))
                    return selfG, groups

                pending = {}
                for b in range(min(PF, NBLK)):
                    pending[b] = emit_block(b)
                for b in range(NBLK):
                    if b + PF < NBLK:
                        pending[b + PF] = emit_block(b + PF)
                    selfG, my_groups = pending.pop(b)
                    U = pu.tile([128, gcol], f32, tag="U")
                    adb = selfG[:, fdim + nheads:fdim + 2 * nheads]
                    # ---- dst logits for every group of this block, up front:
                    # ad_in[e, h] = adb[dstrow(e), h] via matmul against the
                    # prebaked transposed selection matrix (needs only selfG
                    # + static stg, so it runs well before the gathers land)
                    adp = pa.tile([128, NGMAX * GB * nheads], f32, tag="adp")
                    for gi, (s, k, gt_raw, stg, s4) in enumerate(my_groups):
                        for i in range(k):
                            o = (gi * GB + i) * nheads
                            nc.tensor.matmul(
                                adp[:, o:o + nheads],
                                lhsT=stg[:, i * 128:(i + 1) * 128], rhs=adb,
                                start=True, stop=True)
                    # ---- self tile: S = I, ad_e = adb directly ----
                    evs = wp.tile([128, nheads], bf16, tag="ev")
                    nc.vector.tensor_tensor(
                        out=evs[:], in0=selfG[:, fdim:fdim + nheads], in1=adb,
                        op=mybir.AluOpType.add)
                    ev2s = wp.tile([128, nheads], bf16, tag="ev2")
                    nc.vector.tensor_tensor(
                        out=ev2s[:], in0=evs[:],
                        in1=c02[:, 0:1].to_broadcast([128, nheads]),
                        op=mybir.AluOpType.mult)
                    nc.vector.tensor_tensor(out=evs[:], in0=evs[:], in1=ev2s[:],
                                            op=mybir.AluOpType.max)
                    nc.scalar.activation(selfG[:, fdim:fdim + nheads], evs[:],
                                         mybir.ActivationFunctionType.Exp)
                    nc.vector.tensor_tensor(
                        out=selfG[:, 0:fdim].rearrange("p (h o) -> p h o",
                                                       h=nheads),
                        in0=selfG[:, 0:fdim].rearrange("p (h o) -> p h o",
                                                       h=nheads),
                        in1=selfG[:, fdim:fdim + nheads][:, :, None]
                            .to_broadcast([128, nheads, fdim // nheads]),
                        op=mybir.AluOpType.mult)
                    nc.tensor.matmul(U[:], lhsT=idt[:], rhs=selfG[:, 0:gcol],
                                     start=True, stop=(len(my_groups) == 0))
                    # ---- gathered tiles, batched per group ----
                    for gi, (s, k, gt_raw, stg, s4) in enumerate(my_groups):
                        gt = gt_raw[:, :k * rowlen].rearrange(
                            "p (t r) -> p t r", r=rowlen)
                        o = gi * GB * nheads
                        adv = wp.tile([128, GB * nheads], bf16, tag="adv")
                        nc.scalar.copy(adv[:, :k * nheads],
                                       adp[:, o:o + k * nheads])
                        ev = wp.tile([128, GB * nheads], bf16, tag="ev4")
                        nc.vector.tensor_tensor(
                            out=ev[:, :k * nheads].rearrange(
                                "p (t h) -> p t h", t=k),
                            in0=gt[:, :k, fdim:fdim + nheads],
                            in1=adv[:, :k * nheads].rearrange(
                                "p (t h) -> p t h", t=k),
                            op=mybir.AluOpType.add)
                        ev2 = wp.tile([128, GB * nheads], bf16, tag="ev42")
                        nc.vector.tensor_tensor(
                            out=ev2[:, :k * nheads], in0=ev[:, :k * nheads],
                            in1=c02[:, 0:1].to_broadcast([128, k * nheads]),
                            op=mybir.AluOpType.mult)
                        nc.vector.tensor_tensor(
                            out=ev[:, :k * nheads], in0=ev[:, :k * nheads],
                            in1=ev2[:, :k * nheads], op=mybir.AluOpType.max)
                        nc.scalar.activation(
                            gt[:, :k, fdim:fdim + nheads],
                            ev[:, :k * nheads].rearrange("p (t h) -> p t h", t=k),
                            mybir.ActivationFunctionType.Exp)
                        nc.vector.tensor_tensor(
                            out=gt[:, :k, 0:fdim].rearrange(
                                "p t (h o) -> p t h o", h=nheads),
                            in0=gt[:, :k, 0:fdim].rearrange(
                                "p t (h o) -> p t h o", h=nheads),
                            in1=gt[:, :k, fdim:fdim + nheads][:, :, :, None]
                                .to_broadcast([128, k, nheads, fdim // nheads]),
                            op=mybir.AluOpType.mult)
                        last_g = gi == len(my_groups) - 1
                        for i in range(k):
                            nc.tensor.matmul(
                                U[:], lhsT=s4[:, i * 128:(i + 1) * 128],
                                rhs=gt[:, i, 0:gcol],
                                start=False, stop=(last_g and i == k - 1))
                    finish_block(b, U, selfG)
                    if after_block is not None:
                        after_block(b)

            def finish1(b, U, selfG):
                Uc = bp.tile([128, GCOL1], f32, tag="Uc")
                nc.scalar.copy(Uc[:], U[:])
                rec = wp.tile([128, HEADS], f32, tag="rec")
                nc.vector.reciprocal(rec[:], Uc[:, 256:256 + HEADS])
                OB = bp.tile([128, 256], f32, tag="OB")
                nc.vector.tensor_tensor(
                    out=OB[:].rearrange("p (h o) -> p h o", h=HEADS),
                    in0=Uc[:, 0:256].rearrange("p (h o) -> p h o", h=HEADS),
                    in1=rec[:, :, None].to_broadcast([128, HEADS, HID]),
                    op=mybir.AluOpType.mult)
                nc.vector.tensor_tensor(out=OB[:], in0=OB[:], in1=b1t[:],
                                        op=mybir.AluOpType.add)
                # ELU(z) = relu(z) + exp(-relu(-z)) - 1, relu/exp on scalar
                mn = bp.tile([128, 256], f32, tag="mn")
                nc.scalar.activation(mn[:], OB[:],
                                     mybir.ActivationFunctionType.Relu,
                                     scale=-1.0)
                nc.scalar.activation(mn[:], mn[:],
                                     mybir.ActivationFunctionType.Exp,
                                     scale=-1.0)
                nc.scalar.activation(OB[:], OB[:],
                                     mybir.ActivationFunctionType.Relu)
                nc.vector.tensor_tensor(out=OB[:], in0=OB[:], in1=mn[:],
                                        op=mybir.AluOpType.add)
                nc.vector.tensor_tensor(
                    out=OB[:], in0=OB[:],
                    in1=cm1[:, 0:1].to_broadcast([128, 256]),
                    op=mybir.AluOpType.subtract)
                h2p = ph.tile([128, COL2], f32, tag="h2p")
                for kk in range(2):
                    tp = pp.tile([128, 128], f32, tag="scratch")
                    nc.tensor.transpose(tp[:], OB[:, kk * 128:(kk + 1) * 128],
                                        idtf[:])
                    ts_ = wp.tile([128, 128], bf16, tag="ts")
                    nc.scalar.copy(ts_[:], tp[:])
                    nc.tensor.matmul(h2p[:], lhsT=ts_[:],
                                     rhs=w2c[:, kk * COL2:(kk + 1) * COL2],
                                     start=(kk == 0), stop=(kk == 1))
                h2s = bp.tile([128, COL2], bf16, tag="h2s")
                nc.scalar.copy(h2s[:], h2p[:])
                nc.sync.dma_start(l2sh[b * 128:(b + 1) * 128, 0:GCOL2],
                                  h2s[:, 0:GCOL2])
                nc.sync.dma_start(own2[b * 128:(b + 1) * 128, 0:COL2], h2s[:])

            def ag2_after(b):
                for k in range(len(AG_BLKS) - 1):
                    if AG_BLKS[k] == b:
                        ag_chunk(l2sh, l2tabA, l2tabB, k, GCOL2)

            edge_layer(l1tabA, l1tabB, own1, ROW1, COL1, GCOL1, HEADS, 256,
                       finish1, after_block=ag2_after)
            ag_chunk(l2sh, l2tabA, l2tabB, len(AG_BLKS) - 1, GCOL2)

            def finish2(b, U, selfG):
                Uc = bp.tile([128, GCOL2], f32, tag="Uc2")
                nc.scalar.copy(Uc[:], U[:])
                rec = wp.tile([128, 1], f32, tag="rec2")
                nc.vector.reciprocal(rec[:], Uc[:, OUT_F:OUT_F + 1])
                OB = bp.tile([128, OUT_F], f32, tag="OB2")
                nc.vector.tensor_tensor(
                    out=OB[:], in0=Uc[:, 0:OUT_F],
                    in1=rec[:, 0:1].to_broadcast([128, OUT_F]),
                    op=mybir.AluOpType.mult)
                nc.sync.dma_start(out_d[b * 128:(b + 1) * 128, :], OB[:])

            edge_layer(l2tabA, l2tabB, own2, ROW2, COL2, GCOL2, 1, 128,
                       finish2)

    nc.compile()
    return nc


def kernel(x, edge_index, W1, att_src1, att_dst1, b1, W2, att_src2, att_dst2, b2):
    from concourse.bass_utils import run_bass_kernel_spmd

    x = np.asarray(x, dtype=np.float32)
    edge_index = np.asarray(edge_index).astype(np.int64)
    W1 = np.asarray(W1, dtype=np.float32)
    att_src1 = np.asarray(att_src1, dtype=np.float32)
    att_dst1 = np.asarray(att_dst1, dtype=np.float32)
    b1 = np.asarray(b1, dtype=np.float32)
    W2 = np.asarray(W2, dtype=np.float32)
    att_src2 = np.asarray(att_src2, dtype=np.float32)
    att_dst2 = np.asarray(att_dst2, dtype=np.float32)
    b2 = np.asarray(b2, dtype=np.float32)

    plan, idx16, dstrow = _plan(edge_index)
    Ttot = len(plan)
    key = tuple(plan)
    if _cache.get("key") != key:
        _cache["nc"] = _build(plan)
        _cache["key"] = key
    nc = _cache["nc"]

    W1r = W1.reshape(IN_F, HEADS, HID)
    Ws1 = np.einsum("khc,hc->kh", W1r, att_src1).astype(np.float32)
    Wd1 = np.einsum("khc,hc->kh", W1r, att_dst1).astype(np.float32)
    wcat = np.concatenate([W1, Ws1, Wd1], axis=1).astype(BF16)
    Ws2 = (W2 @ att_src2[0]).astype(np.float32)[:, None]
    Wd2 = (W2 @ att_dst2[0]).astype(np.float32)[:, None]
    w2a = np.concatenate([W2, Ws2, Wd2], axis=1).astype(BF16)
    b1row = np.tile(b1[None, :], (128, 1)).astype(np.float32)
    iota = np.tile(np.arange(128, dtype=np.float32)[None, :], (128, 1))
    identity = np.eye(128, dtype=np.float32)

    xp_all = np.zeros((N_CORES, LOCP, IN_F), dtype=np.float32)
    for o in range(N_CORES):
        xp_all[o, :LOC] = x[o * LOC:(o + 1) * LOC]
    # j-major layout: column (j*1024 + o*128 + p) = features of node (o, j*128+p)
    xtf = np.ascontiguousarray(
        xp_all.reshape(N_CORES, NBLK, 128, IN_F).transpose(3, 1, 0, 2)
    ).reshape(IN_F, NT).astype(BF16)

    in_maps = []
    for c in range(N_CORES):
        xp = xp_all[c]
        # stf[d, t*128+e] = 1 iff edge e of tile t lands on dst row d
        stf = (dstrow[c][None, :, :] ==
               np.arange(128, dtype=np.float32)[:, None, None])
        stf = stf.astype(BF16).reshape(128, Ttot * 128)
        # s4f[e, t*128+d] = same selection, edge-major (agg matmul lhsT)
        s4f = (dstrow[c][:, :, None] ==
               np.arange(128, dtype=np.float32)[None, None, :])
        s4f = np.ascontiguousarray(
            s4f.transpose(1, 0, 2)).astype(BF16).reshape(128, Ttot * 128)
        in_maps.append({
            "xT": xtf,
            "xTo": np.ascontiguousarray(xp.T).astype(BF16),
            "wcat": wcat, "w2a": w2a, "b1row": b1row,
            "ident": identity.astype(BF16),
            "identf": identity,
            "idx": _wrap16(idx16[c]),
            "stf": stf.astype(ml_dtypes.float8_e4m3fn),
            "s4f": s4f.astype(ml_dtypes.float8_e4m3fn),
        })

    res = run_bass_kernel_spmd(nc, in_maps, core_ids=list(range(N_CORES)),
                               **_cache.get("run_kwargs", {}))
    _cache["last_result"] = res
    out = np.zeros((N, OUT_F), dtype=np.float32)
    for c in range(N_CORES):
        out[c * LOC:(c + 1) * LOC] = res.results[c]["out"][:LOC]
    return out + b2[None, :]


# revision 65
# speedup vs baseline: 1.0925x; 1.0925x over previous
"""Trainium2 Bass kernel for a 2-layer GAT (nn_LogicGNN): 8-core SPMD.

Sharding: destination nodes across 8 cores (each core owns N/8 dst nodes and
all edges into them -> softmax stats are core-local, no all-reduce). Dense
projections are node-sharded and exchanged with AllGather. Edge phase: per
128-dst-node block, one dense self-loop tile plus packed 128-edge dma_gather
tiles; softmax computed without max-subtraction (logits are O(1) here,
mathematically identical); one PSUM matmul per tile against a 0/1 selection
matrix accumulates softmax denominators and weighted feature sums together.

V2: tables/edge pipeline in bf16 (half the gather bytes, 2x DVE/PE rate);
dst-side logits come from a host-prebaked transposed selection matrix (stf)
streamed sequentially + one small matmul per tile, replacing the per-edge
dst-logit dma_gather (which was index-rate bound on gpsimd) and L2's
per-tile PE transposes; leaky-relu moved to the scalar engine (Lrelu).
"""
import sys
sys.path.insert(0, "/opt/trn_rl_repo")
sys.path.insert(0, "/root/.axon_site")

import numpy as np
import ml_dtypes

BF16 = np.float16

N = 50000
E = 800000
IN_F, HID, OUT_F, HEADS = 128, 64, 128, 4
NEG_SLOPE = 0.2
N_CORES = 8
LOC = 6250                    # real nodes per core
LOCP = 6272                   # padded to 49*128
NBLK = LOCP // 128            # 49 blocks per core
NT = N_CORES * LOCP           # table rows = 50176
HALF = NT // 2                # 25088, int16-safe gather halves
ROW1 = 384                    # L1 table row bf16 elems (768B): [h1 256|as 4|pad]
ROW2 = 256                    # L2 table row bf16 elems (512B): [h2 128|as2 1|pad]
COL1 = 264                    # own1 cols: h 256 | as 4 | ad 4
GCOL1 = 260                   # gathered cols used: h 256 | as 4
COL2 = 130                    # own2 cols: h2 128 | as2 1 | ad2 1
GCOL2 = 129
GB = 8                        # tiles per dma_gather call (1024 idxs)
EPS = 1e-30
AG_SIZES = [1536, 1536, 64, 64, 1536, 1536]   # chunk rows (block-aligned;
AG_STARTS = [0, 1536, 3072, 3136, 3200, 4736]  # the half boundary splits
AG_INBASE = [0, 12288, 24576, 0, 512, 12800]   # block 24 into two 64-row
AG_ISB = [0, 0, 0, 1, 1, 1]                    # chunks)
AG_BLKS = [11, 23, 24, 24, 36, 48]             # last producing block per chunk

_cache = {}


def _plan(edge_index):
    """Host preprocessing. Returns the shared tile plan [(block, half)...] and
    per-core idx16 [C,T,128] (row index within table half) + dstrow [C,T,128]
    (dst position within the 128-node block; 999 for pad lanes)."""
    src = np.concatenate([edge_index[0], np.arange(N, dtype=np.int64)])
    dst = np.concatenate([edge_index[1], np.arange(N, dtype=np.int64)])
    is_added_loop = np.zeros(len(src), dtype=bool)
    is_added_loop[E:] = True                 # only the appended loops go dense
    owner = dst // LOC
    o_ = src // LOC
    l_ = src % LOC
    # chunk-major half-table layout (half A = local rows [0,3136)): each
    # AllGather chunk's output region is contiguous; the last chunk is small
    # so the final ship after the producing phase's tail is cheap
    sz = np.array(AG_SIZES); st = np.array(AG_STARTS)
    ib = np.array(AG_INBASE); isb = np.array(AG_ISB)
    ends = np.cumsum(sz)
    c_ = np.searchsorted(ends, l_, side='right')
    trow = isb[c_] * HALF + ib[c_] + o_ * sz[c_] + (l_ - st[c_])

    per_core = []
    cnt = np.zeros((N_CORES, NBLK, 2), dtype=np.int64)
    for c in range(N_CORES):
        m = (owner == c) & (~is_added_loop)
        ld = (dst[m] - c * LOC).astype(np.int64)
        tr = trow[m]
        blk = ld // 128
        half = (tr >= HALF).astype(np.int64)
        order = np.lexsort((ld, half, blk))
        ld, tr, blk, half = ld[order], tr[order], blk[order], half[order]
        per_core.append((ld, tr, blk, half))
        for b in range(NBLK):
            mb = blk == b
            cnt[c, b, 0] = np.count_nonzero(mb & (half == 0))
            cnt[c, b, 1] = np.count_nonzero(mb & (half == 1))
    tiles = np.ceil(cnt / 128.0).astype(np.int64).max(axis=0)   # [NBLK, 2]

    plan = []
    for b in range(NBLK):
        for h in (0, 1):
            plan.extend([(b, h)] * int(tiles[b, h]))
    Ttot = len(plan)
    idx16 = np.zeros((N_CORES, Ttot, 128), dtype=np.int16)
    dstrow = np.full((N_CORES, Ttot, 128), 999.0, dtype=np.float32)
    for c in range(N_CORES):
        ld, tr, blk, half = per_core[c]
        ti = 0
        for b in range(NBLK):
            for h in (0, 1):
                m = (blk == b) & (half == h)
                lds, trs = ld[m], tr[m]
                k = len(lds)
                for _t in range(int(tiles[b, h])):
                    lo = _t * 128
                    n_here = max(0, min(128, k - lo))
                    if n_here > 0:
                        rel = trs[lo:lo + n_here] - (HALF if h else 0)
                        idx16[c, ti, :n_here] = rel.astype(np.int16)
                        dstrow[c, ti, :n_here] = (
                            lds[lo:lo + n_here] - b * 128).astype(np.float32)
                    ti += 1
    return plan, idx16, dstrow


def _wrap16(idx):
    """[T,128] int16 -> dma_gather wrapped idx layout [128, T*8]."""
    T = idx.shape[0]
    out = np.zeros((128, T * 8), dtype=np.int16)
    for t in range(T):
        blk = idx[t].reshape(8, 16).T
        out[:, t * 8:(t + 1) * 8] = np.tile(blk, (8, 1))
    return out


def _build(plan):
    import concourse.bacc as bacc
    import concourse.mybir as mybir
    from concourse import tile
    from concourse.library_config import mlp

    f32 = mybir.dt.float32
    bf16 = mybir.dt.float16
    Ttot = len(plan)

    nc = bacc.Bacc("TRN2", target_bir_lowering=False, debug=False,
                   num_devices=N_CORES, num_swdge_queues=4)

    xT = nc.dram_tensor("xT", [IN_F, NT], bf16, kind="ExternalInput")
    xTo = nc.dram_tensor("xTo", [IN_F, LOCP], bf16, kind="ExternalInput")
    wcat = nc.dram_tensor("wcat", [IN_F, COL1], bf16, kind="ExternalInput")
    w2a = nc.dram_tensor("w2a", [HEADS * HID, COL2], bf16, kind="ExternalInput")
    b1row = nc.dram_tensor("b1row", [128, 256], f32, kind="ExternalInput")
    s4f_d = nc.dram_tensor("s4f", [128, Ttot * 128], mybir.dt.float8e4, kind="ExternalInput")
    ident = nc.dram_tensor("ident", [128, 128], bf16, kind="ExternalInput")
    identf = nc.dram_tensor("identf", [128, 128], f32, kind="ExternalInput")
    idx_d = nc.dram_tensor("idx", [128, Ttot * 8], mybir.dt.int16, kind="ExternalInput")
    stf_d = nc.dram_tensor("stf", [128, Ttot * 128], mybir.dt.float8e4, kind="ExternalInput")
    out_d = nc.dram_tensor("out", [LOCP, OUT_F], f32, kind="ExternalOutput")

    l1tabA = nc.dram_tensor("l1tabA", [HALF, ROW1], bf16)
    l1tabB = nc.dram_tensor("l1tabB", [HALF, ROW1], bf16)
    l2sh = nc.dram_tensor("l2sh", [LOCP, ROW2], bf16)
    l2tabA = nc.dram_tensor("l2tabA", [HALF, ROW2], bf16, addr_space="Shared")
    l2tabB = nc.dram_tensor("l2tabB", [HALF, ROW2], bf16, addr_space="Shared")
    own1 = nc.dram_tensor("own1", [LOCP, COL1], bf16)
    own2 = nc.dram_tensor("own2", [LOCP, COL2], bf16)

    with tile.TileContext(nc) as tc:
        nc.gpsimd.load_library(mlp)
        with (
            tc.tile_pool(name="const", bufs=1) as cp,
            tc.tile_pool(name="io", bufs=3) as iop,
            tc.tile_pool(name="p0", bufs=2) as p0p,
            tc.tile_pool(name="g", bufs=13) as gp,
            tc.tile_pool(name="stg", bufs=10) as sgp,
            tc.tile_pool(name="s4", bufs=10) as s4p,
            tc.tile_pool(name="work", bufs=3) as wp,
            tc.tile_pool(name="selfg", bufs=8) as sp,
            tc.tile_pool(name="blk", bufs=2) as bp,
            tc.tile_pool(name="ps", bufs=2, space="PSUM") as pp,
            tc.tile_pool(name="psh2", bufs=2, space="PSUM") as ph,
            tc.tile_pool(name="psu", bufs=2, space="PSUM") as pu,
            tc.tile_pool(name="psad", bufs=2, space="PSUM") as pa,
        ):
            wc = cp.tile([128, COL1], bf16)
            nc.sync.dma_start(wc[:], wcat[:])
            w2c = cp.tile([128, 2 * COL2], bf16)
            nc.sync.dma_start(w2c[:, :COL2], w2a[0:128, :])
            nc.sync.dma_start(w2c[:, COL2:], w2a[128:256, :])
            b1t = cp.tile([128, 256], f32)
            nc.sync.dma_start(b1t[:], b1row[:])
            c02 = cp.tile([128, 2], bf16)
            nc.vector.memset(c02[:], NEG_SLOPE)
            cm1 = cp.tile([128, 2], f32)
            nc.vector.memset(cm1[:], 1.0)

            idt = cp.tile([128, 128], bf16)
            nc.sync.dma_start(idt[:], ident[:])
            idtf = cp.tile([128, 128], f32)
            nc.sync.dma_start(idtf[:], identf[:])
            idxs = cp.tile([128, Ttot * 8], mybir.dt.int16)
            nc.sync.dma_start(idxs[:], idx_d[:])


            # ---------- P0 ----------
            def ag_chunk(sh, tabA_, tabB_, k, gcol):
                r0, rows = AG_STARTS[k], AG_SIZES[k]
                tab = tabB_ if AG_ISB[k] else tabA_
                ob = AG_INBASE[k]
                nc.gpsimd.collective_compute(
                    "AllGather", mybir.AluOpType.bypass,
                    ins=[sh[r0:r0 + rows, :]],
                    outs=[tab[ob:ob + 8 * rows, :]],
                    replica_groups=[list(range(N_CORES))],
                )

            # Redundant full-table build: every core computes the whole L1
            # table locally (x is replicated), so there is no layer-1
            # AllGather and half-A gathers can start once half A is written.
            # Chunk-major order; per (chunk, core) the 12 block-rows batch
            # into ONE strided write. Block 24 (straddling the A/B half
            # boundary) is handled separately as two 64-row writes.
            for ci in (0, 1, 4, 5):
                stl, csz, ibase = AG_STARTS[ci], AG_SIZES[ci], AG_INBASE[ci]
                tab = l1tabB if AG_ISB[ci] else l1tabA
                for sc in range(2):
                    j0 = stl // 128 + sc * 6
                    xck = p0p.tile([128, 6 * 1024], bf16, tag="xck")
                    nc.sync.dma_start(xck[:], xT[:, j0 * 1024:(j0 + 6) * 1024])
                    for o in range(N_CORES):
                        hacc = p0p.tile([128, 6 * COL1], bf16, tag="hacc")
                        for jj in range(6):
                            ps = pp.tile([128, COL1], f32, tag="scratch")
                            nc.tensor.matmul(
                                ps[:],
                                lhsT=xck[:, jj * 1024 + o * 128:
                                         jj * 1024 + (o + 1) * 128],
                                rhs=wc[:], start=True, stop=True)
                            nc.scalar.copy(
                                hacc[:, jj * COL1:(jj + 1) * COL1], ps[:])
                        r = ibase + o * csz + sc * 768
                        nc.sync.dma_start(
                            tab[r:r + 768, 0:COL1].rearrange(
                                "(blk p) w -> p blk w", blk=6),
                            hacc[:].rearrange("p (blk w) -> p blk w", w=COL1))
                if ci == 1:
                    # block 24 (rows 3072:3200): A tail + B head
                    x24 = p0p.tile([128, 1024], bf16, tag="x24")
                    nc.sync.dma_start(x24[:], xT[:, 24 * 1024:25 * 1024])
                    for o in range(N_CORES):
                        ps = pp.tile([128, COL1], f32, tag="scratch")
                        nc.tensor.matmul(
                            ps[:], lhsT=x24[:, o * 128:(o + 1) * 128],
                            rhs=wc[:], start=True, stop=True)
                        h24 = p0p.tile([128, COL1], bf16, tag="h24")
                        nc.scalar.copy(h24[:], ps[:])
                        nc.sync.dma_start(
                            l1tabA[AG_INBASE[2] + o * 64:
                                   AG_INBASE[2] + o * 64 + 64, 0:COL1],
                            h24[0:64, :])
                        nc.sync.dma_start(
                            l1tabB[AG_INBASE[3] + o * 64:
                                   AG_INBASE[3] + o * 64 + 64, 0:COL1],
                            h24[64:128, :])

            # own rows for the self path: this core's own 49 blocks
            xfull = cp.tile([128, LOCP], bf16)
            nc.sync.dma_start(xfull[:], xTo[:])
            for j in range(NBLK):
                ps = pp.tile([128, COL1], f32, tag="scratch")
                nc.tensor.matmul(ps[:], lhsT=xfull[:, j * 128:(j + 1) * 128],
                                 rhs=wc[:], start=True, stop=True)
                hrow = iop.tile([128, COL1], bf16, tag="hrow")
                nc.scalar.copy(hrow[:], ps[:])
                nc.sync.dma_start(own1[j * 128:(j + 1) * 128, 0:COL1],
                                  hrow[:])

            def edge_layer(tabA, tabB, ownt, rowlen, colown, gcol, nheads,
                           fdim, finish_block, after_block=None):
                # group spans per block: [(s, k, half), ...]
                spans = [[] for _ in range(NBLK)]
                t0 = 0
                while t0 < Ttot:
                    b0, h0 = plan[t0]
                    t1 = t0
                    while t1 < Ttot and plan[t1] == (b0, h0):
                        t1 += 1
                    for s in range(t0, t1, GB):
                        spans[b0].append((s, min(s + GB, t1) - s, h0))
                    t0 = t1

                NGMAX = max(len(sp_) for sp_ in spans)
                assert NGMAX * GB * nheads <= 512, (NGMAX, nheads)
                qi = [0]
                PF = 6

                def emit_block(b):
                    selfG = sp.tile([128, COL1], bf16, tag="sg")
                    nc.sync.dma_start(selfG[:, :colown],
                                      ownt[b * 128:(b + 1) * 128, 0:colown])
                    groups = []
                    for (s, k, h0) in spans[b]:
                        gt_raw = gp.tile([128, GB * ROW1], bf16, tag="g")
                        gt = gt_raw[:, :k * rowlen].rearrange(
                            "p (t r) -> p t r", r=rowlen)
                        nc.gpsimd.dma_gather(
                            out_ap=gt,
                            in_ap=tabB[:] if h0 else tabA[:],
                            idxs_ap=idxs[:, s * 8:(s + k) * 8],
                            num_idxs=128 * k, num_idxs_reg=128 * k,
                            elem_size=rowlen, queue_num=qi[0] % 4)
                        qi[0] += 1
                        stg = sgp.tile([128, GB * 128], mybir.dt.float8e4, tag="stg")
                        nc.sync.dma_start(stg[:, :k * 128],
                                          stf_d[:, s * 128:(s + k) * 128])
                        s4 = s4p.tile([128, GB * 128], mybir.dt.float8e4, tag="S4")
                        nc.sync.dma_start(s4[:, :k * 128],
                                          s4f_d[:, s * 128:(s + k) * 128])
                        groups.append((s, k, gt_raw, stg, s4))
                    return selfG, groups

                pending = {}
                for b in range(min(PF, NBLK)):
                    pending[b] = emit_block(b)
                for b in range(NBLK):
                    if b + PF < NBLK:
                        pending[b + PF] = emit_block(b + PF)
                    selfG, my_groups = pending.pop(b)
                    U = pu.tile([128, gcol], f32, tag="U")
                    adb = selfG[:, fdim + nheads:fdim + 2 * nheads]
                    # ---- dst logits for every group of this block, up front:
                    # ad_in[e, h] = adb[dstrow(e), h] via matmul against the
                    # prebaked transposed selection matrix (needs only selfG
                    # + static stg, so it runs well before the gathers land)
                    adp = pa.tile([128, NGMAX * GB * nheads], f32, tag="adp")
                    for gi, (s, k, gt_raw, stg, s4) in enumerate(my_groups):
                        for i in range(k):
                            o = (gi * GB + i) * nheads
                            nc.tensor.matmul(
                                adp[:, o:o + nheads],
                                lhsT=stg[:, i * 128:(i + 1) * 128], rhs=adb,
                                start=True, stop=True)
                    # ---- self tile: S = I, ad_e = adb directly ----
                    evs = wp.tile([128, nheads], bf16, tag="ev")
                    nc.vector.tensor_tensor(
                        out=evs[:], in0=selfG[:, fdim:fdim + nheads], in1=adb,
                        op=mybir.AluOpType.add)
                    ev2s = wp.tile([128, nheads], bf16, tag="ev2")
                    nc.vector.tensor_tensor(
                        out=ev2s[:], in0=evs[:],
                        in1=c02[:, 0:1].to_broadcast([128, nheads]),
                        op=mybir.AluOpType.mult)
                    nc.vector.tensor_tensor(out=evs[:], in0=evs[:], in1=ev2s[:],
                                            op=mybir.AluOpType.max)
                    nc.scalar.activation(selfG[:, fdim:fdim + nheads], evs[:],
                                         mybir.ActivationFunctionType.Exp)
                    nc.vector.tensor_tensor(
                        out=selfG[:, 0:fdim].rearrange("p (h o) -> p h o",
                                                       h=nheads),
                        in0=selfG[:, 0:fdim].rearrange("p (h o) -> p h o",
                                                       h=nheads),
                        in1=selfG[:, fdim:fdim + nheads][:, :, None]
                            .to_broadcast([128, nheads, fdim // nheads]),
                        op=mybir.AluOpType.mult)
                    nc.tensor.matmul(U[:], lhsT=idt[:], rhs=selfG[:, 0:gcol],
                                     start=True, stop=(len(my_groups) == 0))
                    # ---- gathered tiles, batched per group ----
                    for gi, (s, k, gt_raw, stg, s4) in enumerate(my_groups):
                        gt = gt_raw[:, :k * rowlen].rearrange(
                            "p (t r) -> p t r", r=rowlen)
                        o = gi * GB * nheads
                        adv = wp.tile([128, GB * nheads], bf16, tag="adv")
                        nc.scalar.copy(adv[:, :k * nheads],
                                       adp[:, o:o + k * nheads])
                        ev = wp.tile([128, GB * nheads], bf16, tag="ev4")
                        nc.vector.tensor_tensor(
                            out=ev[:, :k * nheads].rearrange(
                                "p (t h) -> p t h", t=k),
                            in0=gt[:, :k, fdim:fdim + nheads],
                            in1=adv[:, :k * nheads].rearrange(
                                "p (t h) -> p t h", t=k),
                            op=mybir.AluOpType.add)
                        ev2 = wp.tile([128, GB * nheads], bf16, tag="ev42")
                        nc.vector.tensor_tensor(
                            out=ev2[:, :k * nheads], in0=ev[:, :k * nheads],
                            in1=c02[:, 0:1].to_broadcast([128, k * nheads]),
                            op=mybir.AluOpType.mult)
                        nc.vector.tensor_tensor(
                            out=ev[:, :k * nheads], in0=ev[:, :k * nheads],
                            in1=ev2[:, :k * nheads], op=mybir.AluOpType.max)
                        nc.scalar.activation(
                            gt[:, :k, fdim:fdim + nheads],
                            ev[:, :k * nheads].rearrange("p (t h) -> p t h", t=k),
                            mybir.ActivationFunctionType.Exp)
                        nc.vector.tensor_tensor(
                            out=gt[:, :k, 0:fdim].rearrange(
                                "p t (h o) -> p t h o", h=nheads),
                            in0=gt[:, :k, 0:fdim].rearrange(
                                "p t (h o) -> p t h o", h=nheads),
                            in1=gt[:, :k, fdim:fdim + nheads][:, :, :, None]
                                .to_broadcast([128, k, nheads, fdim // nheads]),
                            op=mybir.AluOpType.mult)
                        last_g = gi == len(my_groups) - 1
                        for i in range(k):
                            nc.tensor.matmul(
                                U[:], lhsT=s4[:, i * 128:(i + 1) * 128],
                                rhs=gt[:, i, 0:gcol],
                                start=False, stop=(last_g and i == k - 1))
                    finish_block(b, U, selfG)
                    if after_block is not None:
                        after_block(b)

            def finish1(b, U, selfG):
                Uc = bp.tile([128, GCOL1], f32, tag="Uc")
                nc.scalar.copy(Uc[:], U[:])
                rec = wp.tile([128, HEADS], f32, tag="rec")
                nc.vector.reciprocal(rec[:], Uc[:, 256:256 + HEADS])
                OB = bp.tile([128, 256], f32, tag="OB")
                nc.vector.tensor_tensor(
                    out=OB[:].rearrange("p (h o) -> p h o", h=HEADS),
                    in0=Uc[:, 0:256].rearrange("p (h o) -> p h o", h=HEADS),
                    in1=rec[:, :, None].to_broadcast([128, HEADS, HID]),
                    op=mybir.AluOpType.mult)
                nc.vector.tensor_tensor(out=OB[:], in0=OB[:], in1=b1t[:],
                                        op=mybir.AluOpType.add)
                # ELU(z) = relu(z) + exp(-relu(-z)) - 1, relu/exp on scalar
                mn = bp.tile([128, 256], f32, tag="mn")
                nc.scalar.activation(mn[:], OB[:],
                                     mybir.ActivationFunctionType.Relu,
                                     scale=-1.0)
                nc.scalar.activation(mn[:], mn[:],
                                     mybir.ActivationFunctionType.Exp,
                                     scale=-1.0)
                nc.scalar.activation(OB[:], OB[:],
                                     mybir.ActivationFunctionType.Relu)
                nc.vector.tensor_tensor(out=OB[:], in0=OB[:], in1=mn[:],
                                        op=mybir.AluOpType.add)
                nc.vector.tensor_tensor(
                    out=OB[:], in0=OB[:],
                    in1=cm1[:, 0:1].to_broadcast([128, 256]),
                    op=mybir.AluOpType.subtract)
                h2p = ph.tile([128, COL2], f32, tag="h2p")
                for kk in range(2):
                    tp = pp.tile([128, 128], f32, tag="scratch")
                    nc.tensor.transpose(tp[:], OB[:, kk * 128:(kk + 1) * 128],
                                        idtf[:])
                    ts_ = wp.tile([128, 128], bf16, tag="ts")
                    nc.scalar.copy(ts_[:], tp[:])
                    nc.tensor.matmul(h2p[:], lhsT=ts_[:],
                                     rhs=w2c[:, kk * COL2:(kk + 1) * COL2],
                                     start=(kk == 0), stop=(kk == 1))
                h2s = bp.tile([128, COL2], bf16, tag="h2s")
                nc.scalar.copy(h2s[:], h2p[:])
                nc.sync.dma_start(l2sh[b * 128:(b + 1) * 128, 0:GCOL2],
                                  h2s[:, 0:GCOL2])
                nc.sync.dma_start(own2[b * 128:(b + 1) * 128, 0:COL2], h2s[:])

            def ag2_after(b):
                for k in range(len(AG_BLKS) - 1):
                    if AG_BLKS[k] == b:
                        ag_chunk(l2sh, l2tabA, l2tabB, k, GCOL2)

            edge_layer(l1tabA, l1tabB, own1, ROW1, COL1, GCOL1, HEADS, 256,
                       finish1, after_block=ag2_after)
            ag_chunk(l2sh, l2tabA, l2tabB, len(AG_BLKS) - 1, GCOL2)

            def finish2(b, U, selfG):
                Uc = bp.tile([128, GCOL2], f32, tag="Uc2")
                nc.scalar.copy(Uc[:], U[:])
                rec = wp.tile([128, 1], f32, tag="rec2")
                nc.vector.reciprocal(rec[:], Uc[:, OUT_F:OUT_F + 1])
                OB = bp.tile([128, OUT_F], f32, tag="OB2")
                nc.vector.tensor_tensor(
                    out=OB[:], in0=Uc[:, 0:OUT_F],
                    in1=rec[:, 0:1].to_broadcast([128, OUT_F]),
                    op=mybir.AluOpType.mult)
                nc.sync.dma_start(out_d[b * 128:(b + 1) * 128, :], OB[:])

            edge_layer(l2tabA, l2tabB, own2, ROW2, COL2, GCOL2, 1, 128,
                       finish2)

    nc.compile()
    return nc


def kernel(x, edge_index, W1, att_src1, att_dst1, b1, W2, att_src2, att_dst2, b2):
    from concourse.bass_utils import run_bass_kernel_spmd

    x = np.asarray(x, dtype=np.float32)
    edge_index = np.asarray(edge_index).astype(np.int64)
    W1 = np.asarray(W1, dtype=np.float32)
    att_src1 = np.asarray(att_src1, dtype=np.float32)
    att_dst1 = np.asarray(att_dst1, dtype=np.float32)
    b1 = np.asarray(b1, dtype=np.float32)
    W2 = np.asarray(W2, dtype=np.float32)
    att_src2 = np.asarray(att_src2, dtype=np.float32)
    att_dst2 = np.asarray(att_dst2, dtype=np.float32)
    b2 = np.asarray(b2, dtype=np.float32)

    plan, idx16, dstrow = _plan(edge_index)
    Ttot = len(plan)
    key = tuple(plan)
    if _cache.get("key") != key:
        _cache["nc"] = _build(plan)
        _cache["key"] = key
    nc = _cache["nc"]

    W1r = W1.reshape(IN_F, HEADS, HID)
    Ws1 = np.einsum("khc,hc->kh", W1r, att_src1).astype(np.float32)
    Wd1 = np.einsum("khc,hc->kh", W1r, att_dst1).astype(np.float32)
    wcat = np.concatenate([W1, Ws1, Wd1], axis=1).astype(BF16)
    Ws2 = (W2 @ att_src2[0]).astype(np.float32)[:, None]
    Wd2 = (W2 @ att_dst2[0]).astype(np.float32)[:, None]
    w2a = np.concatenate([W2, Ws2, Wd2], axis=1).astype(BF16)
    b1row = np.tile(b1[None, :], (128, 1)).astype(np.float32)
    iota = np.tile(np.arange(128, dtype=np.float32)[None, :], (128, 1))
    identity = np.eye(128, dtype=np.float32)

    xp_all = np.zeros((N_CORES, LOCP, IN_F), dtype=np.float32)
    for o in range(N_CORES):
        xp_all[o, :LOC] = x[o * LOC:(o + 1) * LOC]
    # j-major layout: column (j*1024 + o*128 + p) = features of node (o, j*128+p)
    xtf = np.ascontiguousarray(
        xp_all.reshape(N_CORES, NBLK, 128, IN_F).transpose(3, 1, 0, 2)
    ).reshape(IN_F, NT).astype(BF16)

    in_maps = []
    for c in range(N_CORES):
        xp = xp_all[c]
        # stf[d, t*128+e] = 1 iff edge e of tile t lands on dst row d
        stf = (dstrow[c][None, :, :] ==
               np.arange(128, dtype=np.float32)[:, None, None])
        stf = stf.astype(BF16).reshape(128, Ttot * 128)
        # s4f[e, t*128+d] = same selection, edge-major (agg matmul lhsT)
        s4f = (dstrow[c][:, :, None] ==
               np.arange(128, dtype=np.float32)[None, None, :])
        s4f = np.ascontiguousarray(
            s4f.transpose(1, 0, 2)).astype(BF16).reshape(128, Ttot * 128)
        in_maps.append({
            "xT": xtf,
            "xTo": np.ascontiguousarray(xp.T).astype(BF16),
            "wcat": wcat, "w2a": w2a, "b1row": b1row,
            "ident": identity.astype(BF16),
            "identf": identity,
            "idx": _wrap16(idx16[c]),
            "stf": stf.astype(ml_dtypes.float8_e4m3fn),
            "s4f": s4f.astype(ml_dtypes.float8_e4m3fn),
        })

    res = run_bass_kernel_spmd(nc, in_maps, core_ids=list(range(N_CORES)),
                               **_cache.get("run_kwargs", {}))
    _cache["last_result"] = res
    out = np.zeros((N, OUT_F), dtype=np.float32)
    for c in range(N_CORES):
        out[c * LOC:(c + 1) * LOC] = res.results[c]["out"][:LOC]
    return out + b2[None, :]


# revision 67
# speedup vs baseline: 1.1347x; 1.0386x over previous
"""Trainium2 Bass kernel for a 2-layer GAT (nn_LogicGNN): 8-core SPMD.

Sharding: destination nodes across 8 cores (each core owns N/8 dst nodes and
all edges into them -> softmax stats are core-local, no all-reduce). Dense
projections are node-sharded and exchanged with AllGather. Edge phase: per
128-dst-node block, one dense self-loop tile plus packed 128-edge dma_gather
tiles; softmax computed without max-subtraction (logits are O(1) here,
mathematically identical); one PSUM matmul per tile against a 0/1 selection
matrix accumulates softmax denominators and weighted feature sums together.

V2: tables/edge pipeline in bf16 (half the gather bytes, 2x DVE/PE rate);
dst-side logits come from a host-prebaked transposed selection matrix (stf)
streamed sequentially + one small matmul per tile, replacing the per-edge
dst-logit dma_gather (which was index-rate bound on gpsimd) and L2's
per-tile PE transposes; leaky-relu moved to the scalar engine (Lrelu).
"""
import sys
sys.path.insert(0, "/opt/trn_rl_repo")
sys.path.insert(0, "/root/.axon_site")

import numpy as np
import ml_dtypes

BF16 = np.float16

N = 50000
E = 800000
IN_F, HID, OUT_F, HEADS = 128, 64, 128, 4
NEG_SLOPE = 0.2
N_CORES = 8
LOC = 6250                    # real nodes per core
LOCP = 6272                   # padded to 49*128
NBLK = LOCP // 128            # 49 blocks per core
NT = N_CORES * LOCP           # table rows = 50176
HALF = NT // 2                # 25088, int16-safe gather halves
ROW1 = 384                    # L1 table row bf16 elems (768B): [h1 256|as 4|pad]
ROW2 = 256                    # L2 table row bf16 elems (512B): [h2 128|as2 1|pad]
COL1 = 264                    # own1 cols: h 256 | as 4 | ad 4
GCOL1 = 260                   # gathered cols used: h 256 | as 4
COL2 = 130                    # own2 cols: h2 128 | as2 1 | ad2 1
GCOL2 = 129
GB = 8                        # tiles per dma_gather call (1024 idxs)
EPS = 1e-30
AG_SIZES = [1568, 1568, 1568, 1568]        # AllGather chunk rows
AG_STARTS = [0, 1568, 3136, 4704]
AG_INBASE = [0, 12544, 0, 12544]           # output base within the half-table
AG_ISB = [0, 0, 1, 1]                      # chunk belongs to half B
AG_BLKS = [12, 24, 36, 48]                 # last producing block per chunk

_cache = {}


def _plan(edge_index):
    """Host preprocessing. Returns the shared tile plan [(block, half)...] and
    per-core idx16 [C,T,128] (row index within table half) + dstrow [C,T,128]
    (dst position within the 128-node block; 999 for pad lanes)."""
    src = np.concatenate([edge_index[0], np.arange(N, dtype=np.int64)])
    dst = np.concatenate([edge_index[1], np.arange(N, dtype=np.int64)])
    is_added_loop = np.zeros(len(src), dtype=bool)
    is_added_loop[E:] = True                 # only the appended loops go dense
    owner = dst // LOC
    o_ = src // LOC
    l_ = src % LOC
    # chunk-major half-table layout (half A = local rows [0,3136)): each
    # AllGather chunk's output region is contiguous; the last chunk is small
    # so the final ship after the producing phase's tail is cheap
    sz = np.array(AG_SIZES); st = np.array(AG_STARTS)
    ib = np.array(AG_INBASE); isb = np.array(AG_ISB)
    ends = np.cumsum(sz)
    c_ = np.searchsorted(ends, l_, side='right')
    trow = isb[c_] * HALF + ib[c_] + o_ * sz[c_] + (l_ - st[c_])

    per_core = []
    cnt = np.zeros((N_CORES, NBLK, 2), dtype=np.int64)
    for c in range(N_CORES):
        m = (owner == c) & (~is_added_loop)
        ld = (dst[m] - c * LOC).astype(np.int64)
        tr = trow[m]
        blk = ld // 128
        half = (tr >= HALF).astype(np.int64)
        order = np.lexsort((ld, half, blk))
        ld, tr, blk, half = ld[order], tr[order], blk[order], half[order]
        per_core.append((ld, tr, blk, half))
        for b in range(NBLK):
            mb = blk == b
            cnt[c, b, 0] = np.count_nonzero(mb & (half == 0))
            cnt[c, b, 1] = np.count_nonzero(mb & (half == 1))
    tiles = np.ceil(cnt / 128.0).astype(np.int64).max(axis=0)   # [NBLK, 2]

    plan = []
    for b in range(NBLK):
        for h in (0, 1):
            plan.extend([(b, h)] * int(tiles[b, h]))
    Ttot = len(plan)
    idx16 = np.zeros((N_CORES, Ttot, 128), dtype=np.int16)
    dstrow = np.full((N_CORES, Ttot, 128), 999.0, dtype=np.float32)
    for c in range(N_CORES):
        ld, tr, blk, half = per_core[c]
        ti = 0
        for b in range(NBLK):
            for h in (0, 1):
                m = (blk == b) & (half == h)
                lds, trs = ld[m], tr[m]
                k = len(lds)
                for _t in range(int(tiles[b, h])):
                    lo = _t * 128
                    n_here = max(0, min(128, k - lo))
                    if n_here > 0:
                        rel = trs[lo:lo + n_here] - (HALF if h else 0)
                        idx16[c, ti, :n_here] = rel.astype(np.int16)
                        dstrow[c, ti, :n_here] = (
                            lds[lo:lo + n_here] - b * 128).astype(np.float32)
                    ti += 1
    return plan, idx16, dstrow


def _wrap16(idx):
    """[T,128] int16 -> dma_gather wrapped idx layout [128, T*8]."""
    T = idx.shape[0]
    out = np.zeros((128, T * 8), dtype=np.int16)
    for t in range(T):
        blk = idx[t].reshape(8, 16).T
        out[:, t * 8:(t + 1) * 8] = np.tile(blk, (8, 1))
    return out


def _build(plan):
    import concourse.bacc as bacc
    import concourse.mybir as mybir
    from concourse import tile
    from concourse.library_config import mlp

    f32 = mybir.dt.float32
    bf16 = mybir.dt.float16
    Ttot = len(plan)

    nc = bacc.Bacc("TRN2", target_bir_lowering=False, debug=False,
                   num_devices=N_CORES, num_swdge_queues=4)

    xT = nc.dram_tensor("xT", [IN_F, LOCP], bf16, kind="ExternalInput")
    wcat = nc.dram_tensor("wcat", [IN_F, COL1], bf16, kind="ExternalInput")
    w2a = nc.dram_tensor("w2a", [HEADS * HID, COL2], bf16, kind="ExternalInput")
    b1row = nc.dram_tensor("b1row", [128, 256], f32, kind="ExternalInput")
    s4f_d = nc.dram_tensor("s4f", [128, Ttot * 128], mybir.dt.float8e4, kind="ExternalInput")
    ident = nc.dram_tensor("ident", [128, 128], bf16, kind="ExternalInput")
    identf = nc.dram_tensor("identf", [128, 128], f32, kind="ExternalInput")
    idx_d = nc.dram_tensor("idx", [128, Ttot * 8], mybir.dt.int16, kind="ExternalInput")
    stf_d = nc.dram_tensor("stf", [128, Ttot * 128], mybir.dt.float8e4, kind="ExternalInput")
    out_d = nc.dram_tensor("out", [LOCP, OUT_F], f32, kind="ExternalOutput")

    l1sh = nc.dram_tensor("l1sh", [LOCP, ROW1], bf16)
    l1tabA = nc.dram_tensor("l1tabA", [HALF, ROW1], bf16, addr_space="Shared")
    l1tabB = nc.dram_tensor("l1tabB", [HALF, ROW1], bf16, addr_space="Shared")
    l2sh = nc.dram_tensor("l2sh", [LOCP, ROW2], bf16)
    l2tabA = nc.dram_tensor("l2tabA", [HALF, ROW2], bf16, addr_space="Shared")
    l2tabB = nc.dram_tensor("l2tabB", [HALF, ROW2], bf16, addr_space="Shared")
    own1 = nc.dram_tensor("own1", [LOCP, COL1], bf16)
    own2 = nc.dram_tensor("own2", [LOCP, COL2], bf16)

    with tile.TileContext(nc) as tc:
        nc.gpsimd.load_library(mlp)
        with (
            tc.tile_pool(name="const", bufs=1) as cp,
            tc.tile_pool(name="io", bufs=3) as iop,
            tc.tile_pool(name="g", bufs=14) as gp,
            tc.tile_pool(name="stg", bufs=10) as sgp,
            tc.tile_pool(name="s4", bufs=10) as s4p,
            tc.tile_pool(name="work", bufs=3) as wp,
            tc.tile_pool(name="selfg", bufs=8) as sp,
            tc.tile_pool(name="blk", bufs=2) as bp,
            tc.tile_pool(name="ps", bufs=2, space="PSUM") as pp,
            tc.tile_pool(name="psh2", bufs=2, space="PSUM") as ph,
            tc.tile_pool(name="psu", bufs=2, space="PSUM") as pu,
            tc.tile_pool(name="psad", bufs=2, space="PSUM") as pa,
        ):
            wc = cp.tile([128, COL1], bf16)
            nc.sync.dma_start(wc[:], wcat[:])
            w2c = cp.tile([128, 2 * COL2], bf16)
            nc.sync.dma_start(w2c[:, :COL2], w2a[0:128, :])
            nc.sync.dma_start(w2c[:, COL2:], w2a[128:256, :])
            b1t = cp.tile([128, 256], f32)
            nc.sync.dma_start(b1t[:], b1row[:])
            c02 = cp.tile([128, 2], bf16)
            nc.vector.memset(c02[:], NEG_SLOPE)
            cm1 = cp.tile([128, 2], f32)
            nc.vector.memset(cm1[:], 1.0)

            idt = cp.tile([128, 128], bf16)
            nc.sync.dma_start(idt[:], ident[:])
            idtf = cp.tile([128, 128], f32)
            nc.sync.dma_start(idtf[:], identf[:])
            idxs = cp.tile([128, Ttot * 8], mybir.dt.int16)
            nc.sync.dma_start(idxs[:], idx_d[:])


            # ---------- P0 ----------
            def ag_chunk(sh, tabA_, tabB_, k, gcol):
                r0, rows = AG_STARTS[k], AG_SIZES[k]
                tab = tabB_ if AG_ISB[k] else tabA_
                ob = AG_INBASE[k]
                nc.gpsimd.collective_compute(
                    "AllGather", mybir.AluOpType.bypass,
                    ins=[sh[r0:r0 + rows, :]],
                    outs=[tab[ob:ob + 8 * rows, :]],
                    replica_groups=[list(range(N_CORES))],
                )

            xfull = cp.tile([128, LOCP], bf16)
            nc.sync.dma_start(xfull[:], xT[:])
            for j in range(NBLK):
                ps = pp.tile([128, COL1], f32, tag="scratch")
                nc.tensor.matmul(ps[:], lhsT=xfull[:, j * 128:(j + 1) * 128],
                                 rhs=wc[:], start=True, stop=True)
                hrow = iop.tile([128, COL1], bf16, tag="hrow")
                nc.scalar.copy(hrow[:], ps[:])
                nc.scalar.dma_start(l1sh[j * 128:(j + 1) * 128, 0:GCOL1],
                                    hrow[:, 0:GCOL1])
                nc.scalar.dma_start(own1[j * 128:(j + 1) * 128, 0:COL1],
                                    hrow[:])
                for k in range(len(AG_BLKS)):
                    if AG_BLKS[k] == j:
                        ag_chunk(l1sh, l1tabA, l1tabB, k, GCOL1)

            def edge_layer(tabA, tabB, ownt, rowlen, colown, gcol, nheads,
                           fdim, finish_block, after_block=None):
                # group spans per block: [(s, k, half), ...]
                spans = [[] for _ in range(NBLK)]
                t0 = 0
                while t0 < Ttot:
                    b0, h0 = plan[t0]
                    t1 = t0
                    while t1 < Ttot and plan[t1] == (b0, h0):
                        t1 += 1
                    for s in range(t0, t1, GB):
                        spans[b0].append((s, min(s + GB, t1) - s, h0))
                    t0 = t1

                NGMAX = max(len(sp_) for sp_ in spans)
                assert NGMAX * GB * nheads <= 512, (NGMAX, nheads)
                qi = [0]
                PF = 6

                def emit_block(b):
                    selfG = sp.tile([128, COL1], bf16, tag="sg")
                    nc.sync.dma_start(selfG[:, :colown],
                                      ownt[b * 128:(b + 1) * 128, 0:colown])
                    groups = []
                    for (s, k, h0) in spans[b]:
                        gt_raw = gp.tile([128, GB * ROW1], bf16, tag="g")
                        gt = gt_raw[:, :k * rowlen].rearrange(
                            "p (t r) -> p t r", r=rowlen)
                        nc.gpsimd.dma_gather(
                            out_ap=gt,
                            in_ap=tabB[:] if h0 else tabA[:],
                            idxs_ap=idxs[:, s * 8:(s + k) * 8],
                            num_idxs=128 * k, num_idxs_reg=128 * k,
                            elem_size=rowlen, queue_num=qi[0] % 4)
                        qi[0] += 1
                        stg = sgp.tile([128, GB * 128], mybir.dt.float8e4, tag="stg")
                        nc.sync.dma_start(stg[:, :k * 128],
                                          stf_d[:, s * 128:(s + k) * 128])
                        s4 = s4p.tile([128, GB * 128], mybir.dt.float8e4, tag="S4")
                        nc.sync.dma_start(s4[:, :k * 128],
                                          s4f_d[:, s * 128:(s + k) * 128])
                        groups.append((s, k, gt_raw, stg, s4))
                    return selfG, groups

                pending = {}
                for b in range(min(PF, NBLK)):
                    pending[b] = emit_block(b)
                for b in range(NBLK):
                    if b + PF < NBLK:
                        pending[b + PF] = emit_block(b + PF)
                    selfG, my_groups = pending.pop(b)
                    U = pu.tile([128, gcol], f32, tag="U")
                    adb = selfG[:, fdim + nheads:fdim + 2 * nheads]
                    # ---- dst logits for every group of this block, up front:
                    # ad_in[e, h] = adb[dstrow(e), h] via matmul against the
                    # prebaked transposed selection matrix (needs only selfG
                    # + static stg, so it runs well before the gathers land)
                    adp = pa.tile([128, NGMAX * GB * nheads], f32, tag="adp")
                    for gi, (s, k, gt_raw, stg, s4) in enumerate(my_groups):
                        for i in range(k):
                            o = (gi * GB + i) * nheads
                            nc.tensor.matmul(
                                adp[:, o:o + nheads],
                                lhsT=stg[:, i * 128:(i + 1) * 128], rhs=adb,
                                start=True, stop=True)
                    # ---- self tile: S = I, ad_e = adb directly ----
                    evs = wp.tile([128, nheads], bf16, tag="ev")
                    nc.vector.tensor_tensor(
                        out=evs[:], in0=selfG[:, fdim:fdim + nheads], in1=adb,
                        op=mybir.AluOpType.add)
                    ev2s = wp.tile([128, nheads], bf16, tag="ev2")
                    nc.vector.tensor_tensor(
                        out=ev2s[:], in0=evs[:],
                        in1=c02[:, 0:1].to_broadcast([128, nheads]),
                        op=mybir.AluOpType.mult)
                    nc.vector.tensor_tensor(out=evs[:], in0=evs[:], in1=ev2s[:],
                                            op=mybir.AluOpType.max)
                    nc.scalar.activation(selfG[:, fdim:fdim + nheads], evs[:],
                                         mybir.ActivationFunctionType.Exp)
                    nc.vector.tensor_tensor(
                        out=selfG[:, 0:fdim].rearrange("p (h o) -> p h o",
                                                       h=nheads),
                        in0=selfG[:, 0:fdim].rearrange("p (h o) -> p h o",
                                                       h=nheads),
                        in1=selfG[:, fdim:fdim + nheads][:, :, None]
                            .to_broadcast([128, nheads, fdim // nheads]),
                        op=mybir.AluOpType.mult)
                    nc.tensor.matmul(U[:], lhsT=idt[:], rhs=selfG[:, 0:gcol],
                                     start=True, stop=(len(my_groups) == 0))
                    # ---- gathered tiles, batched per group ----
                    for gi, (s, k, gt_raw, stg, s4) in enumerate(my_groups):
                        gt = gt_raw[:, :k * rowlen].rearrange(
                            "p (t r) -> p t r", r=rowlen)
                        o = gi * GB * nheads
                        adv = wp.tile([128, GB * nheads], bf16, tag="adv")
                        nc.scalar.copy(adv[:, :k * nheads],
                                       adp[:, o:o + k * nheads])
                        ev = wp.tile([128, GB * nheads], bf16, tag="ev4")
                        nc.vector.tensor_tensor(
                            out=ev[:, :k * nheads].rearrange(
                                "p (t h) -> p t h", t=k),
                            in0=gt[:, :k, fdim:fdim + nheads],
                            in1=adv[:, :k * nheads].rearrange(
                                "p (t h) -> p t h", t=k),
                            op=mybir.AluOpType.add)
                        ev2 = wp.tile([128, GB * nheads], bf16, tag="ev42")
                        nc.vector.tensor_tensor(
                            out=ev2[:, :k * nheads], in0=ev[:, :k * nheads],
                            in1=c02[:, 0:1].to_broadcast([128, k * nheads]),
                            op=mybir.AluOpType.mult)
                        nc.vector.tensor_tensor(
                            out=ev[:, :k * nheads], in0=ev[:, :k * nheads],
                            in1=ev2[:, :k * nheads], op=mybir.AluOpType.max)
                        nc.scalar.activation(
                            gt[:, :k, fdim:fdim + nheads],
                            ev[:, :k * nheads].rearrange("p (t h) -> p t h", t=k),
                            mybir.ActivationFunctionType.Exp)
                        nc.vector.tensor_tensor(
                            out=gt[:, :k, 0:fdim].rearrange(
                                "p t (h o) -> p t h o", h=nheads),
                            in0=gt[:, :k, 0:fdim].rearrange(
                                "p t (h o) -> p t h o", h=nheads),
                            in1=gt[:, :k, fdim:fdim + nheads][:, :, :, None]
                                .to_broadcast([128, k, nheads, fdim // nheads]),
                            op=mybir.AluOpType.mult)
                        last_g = gi == len(my_groups) - 1
                        for i in range(k):
                            nc.tensor.matmul(
                                U[:], lhsT=s4[:, i * 128:(i + 1) * 128],
                                rhs=gt[:, i, 0:gcol],
                                start=False, stop=(last_g and i == k - 1))
                    finish_block(b, U, selfG)
                    if after_block is not None:
                        after_block(b)

            def finish1(b, U, selfG):
                Uc = bp.tile([128, GCOL1], f32, tag="Uc")
                nc.scalar.copy(Uc[:], U[:])
                rec = wp.tile([128, HEADS], f32, tag="rec")
                nc.vector.reciprocal(rec[:], Uc[:, 256:256 + HEADS])
                OB = bp.tile([128, 256], f32, tag="OB")
                nc.vector.tensor_tensor(
                    out=OB[:].rearrange("p (h o) -> p h o", h=HEADS),
                    in0=Uc[:, 0:256].rearrange("p (h o) -> p h o", h=HEADS),
                    in1=rec[:, :, None].to_broadcast([128, HEADS, HID]),
                    op=mybir.AluOpType.mult)
                nc.vector.tensor_tensor(out=OB[:], in0=OB[:], in1=b1t[:],
                                        op=mybir.AluOpType.add)
                # ELU(z) = relu(z) + exp(-relu(-z)) - 1, relu/exp on scalar
                mn = bp.tile([128, 256], f32, tag="mn")
                nc.scalar.activation(mn[:], OB[:],
                                     mybir.ActivationFunctionType.Relu,
                                     scale=-1.0)
                nc.scalar.activation(mn[:], mn[:],
                                     mybir.ActivationFunctionType.Exp,
                                     scale=-1.0)
                nc.scalar.activation(OB[:], OB[:],
                                     mybir.ActivationFunctionType.Relu)
                nc.vector.tensor_tensor(out=OB[:], in0=OB[:], in1=mn[:],
                                        op=mybir.AluOpType.add)
                nc.vector.tensor_tensor(
                    out=OB[:], in0=OB[:],
                    in1=cm1[:, 0:1].to_broadcast([128, 256]),
                    op=mybir.AluOpType.subtract)
                h2p = ph.tile([128, COL2], f32, tag="h2p")
                for kk in range(2):
                    tp = pp.tile([128, 128], f32, tag="scratch")
                    nc.tensor.transpose(tp[:], OB[:, kk * 128:(kk + 1) * 128],
                                        idtf[:])
                    ts_ = wp.tile([128, 128], bf16, tag="ts")
                    nc.scalar.copy(ts_[:], tp[:])
                    nc.tensor.matmul(h2p[:], lhsT=ts_[:],
                                     rhs=w2c[:, kk * COL2:(kk + 1) * COL2],
                                     start=(kk == 0), stop=(kk == 1))
                h2s = bp.tile([128, COL2], bf16, tag="h2s")
                nc.scalar.copy(h2s[:], h2p[:])
                nc.sync.dma_start(l2sh[b * 128:(b + 1) * 128, 0:GCOL2],
                                  h2s[:, 0:GCOL2])
                nc.sync.dma_start(own2[b * 128:(b + 1) * 128, 0:COL2], h2s[:])

            def ag2_after(b):
                for k in range(len(AG_BLKS) - 1):
                    if AG_BLKS[k] == b:
                        ag_chunk(l2sh, l2tabA, l2tabB, k, GCOL2)

            edge_layer(l1tabA, l1tabB, own1, ROW1, COL1, GCOL1, HEADS, 256,
                       finish1, after_block=ag2_after)
            ag_chunk(l2sh, l2tabA, l2tabB, len(AG_BLKS) - 1, GCOL2)

            def finish2(b, U, selfG):
                Uc = bp.tile([128, GCOL2], f32, tag="Uc2")
                nc.scalar.copy(Uc[:], U[:])
                rec = wp.tile([128, 1], f32, tag="rec2")
                nc.vector.reciprocal(rec[:], Uc[:, OUT_F:OUT_F + 1])
                OB = bp.tile([128, OUT_F], f32, tag="OB2")
                nc.vector.tensor_tensor(
                    out=OB[:], in0=Uc[:, 0:OUT_F],
                    in1=rec[:, 0:1].to_broadcast([128, OUT_F]),
                    op=mybir.AluOpType.mult)
                nc.sync.dma_start(out_d[b * 128:(b + 1) * 128, :], OB[:])

            edge_layer(l2tabA, l2tabB, own2, ROW2, COL2, GCOL2, 1, 128,
                       finish2)

    nc.compile()
    return nc


def kernel(x, edge_index, W1, att_src1, att_dst1, b1, W2, att_src2, att_dst2, b2):
    from concourse.bass_utils import run_bass_kernel_spmd

    x = np.asarray(x, dtype=np.float32)
    edge_index = np.asarray(edge_index).astype(np.int64)
    W1 = np.asarray(W1, dtype=np.float32)
    att_src1 = np.asarray(att_src1, dtype=np.float32)
    att_dst1 = np.asarray(att_dst1, dtype=np.float32)
    b1 = np.asarray(b1, dtype=np.float32)
    W2 = np.asarray(W2, dtype=np.float32)
    att_src2 = np.asarray(att_src2, dtype=np.float32)
    att_dst2 = np.asarray(att_dst2, dtype=np.float32)
    b2 = np.asarray(b2, dtype=np.float32)

    plan, idx16, dstrow = _plan(edge_index)
    Ttot = len(plan)
    key = tuple(plan)
    if _cache.get("key") != key:
        _cache["nc"] = _build(plan)
        _cache["key"] = key
    nc = _cache["nc"]

    W1r = W1.reshape(IN_F, HEADS, HID)
    Ws1 = np.einsum("khc,hc->kh", W1r, att_src1).astype(np.float32)
    Wd1 = np.einsum("khc,hc->kh", W1r, att_dst1).astype(np.float32)
    wcat = np.concatenate([W1, Ws1, Wd1], axis=1).astype(BF16)
    Ws2 = (W2 @ att_src2[0]).astype(np.float32)[:, None]
    Wd2 = (W2 @ att_dst2[0]).astype(np.float32)[:, None]
    w2a = np.concatenate([W2, Ws2, Wd2], axis=1).astype(BF16)
    b1row = np.tile(b1[None, :], (128, 1)).astype(np.float32)
    iota = np.tile(np.arange(128, dtype=np.float32)[None, :], (128, 1))
    identity = np.eye(128, dtype=np.float32)

    in_maps = []
    for c in range(N_CORES):
        xp = np.zeros((LOCP, IN_F), dtype=np.float32)
        xp[:LOC] = x[c * LOC:(c + 1) * LOC]
        # stf[d, t*128+e] = 1 iff edge e of tile t lands on dst row d
        stf = (dstrow[c][None, :, :] ==
               np.arange(128, dtype=np.float32)[:, None, None])
        stf = stf.astype(BF16).reshape(128, Ttot * 128)
        # s4f[e, t*128+d] = same selection, edge-major (agg matmul lhsT)
        s4f = (dstrow[c][:, :, None] ==
               np.arange(128, dtype=np.float32)[None, None, :])
        s4f = np.ascontiguousarray(
            s4f.transpose(1, 0, 2)).astype(BF16).reshape(128, Ttot * 128)
        in_maps.append({
            "xT": np.ascontiguousarray(xp.T).astype(BF16),
            "wcat": wcat, "w2a": w2a, "b1row": b1row,
            "ident": identity.astype(BF16),
            "identf": identity,
            "idx": _wrap16(idx16[c]),
            "stf": stf.astype(ml_dtypes.float8_e4m3fn),
            "s4f": s4f.astype(ml_dtypes.float8_e4m3fn),
        })

    res = run_bass_kernel_spmd(nc, in_maps, core_ids=list(range(N_CORES)),
                               **_cache.get("run_kwargs", {}))
    _cache["last_result"] = res
    out = np.zeros((N, OUT_F), dtype=np.float32)
    for c in range(N_CORES):
        out[c * LOC:(c + 1) * LOC] = res.results[c]["out"][:LOC]
    return out + b2[None, :]
